# revision 1
# baseline (speedup 1.0000x reference)
"""AffineTransformer Trainium kernel: host planner + bass program builder.

Per-voxel sampling via one gpsimd.ap_gather (d=4) from quad-expanded
z-plane windows (16 planes per core group); tent weights over planes do
the z-selection/interpolation; x/y lerp folds on DVE; PE block-reduce;
segment scatter (indirect DMA) to a transposed per-core output.
"""
import sys
sys.path.insert(0, "/opt/trn_rl_repo")
import numpy as np
from contextlib import ExitStack

import concourse.bass as bass
import concourse.bacc as bacc
import concourse.tile as tile
from concourse import mybir

f32 = mybir.dt.float32
i16 = mybir.dt.int16
u32 = mybir.dt.uint32
A = mybir.AluOpType
AF = mybir.ActivationFunctionType

NCORES = 8
MAGIC = float(np.float32(12582912.0))


def compose_mats_np(affine, scale, translate, shear):
    """float32 mirror of reference._compose_mats."""
    ft = np.float32
    B = affine.shape[0]
    one = np.ones(B, ft)
    zero = np.zeros(B, ft)
    cx, sx = np.cos(affine[:, 0]), np.sin(affine[:, 0])
    cy, sy = np.cos(affine[:, 1]), np.sin(affine[:, 1])
    cz, sz = np.cos(affine[:, 2]), np.sin(affine[:, 2])

    def mk(rows):
        M = np.stack([np.stack(r, axis=1) for r in rows], axis=1)
        return np.swapaxes(M, 1, 2)

    rot_x = mk([[one, zero, zero], [zero, cx, -sx], [zero, sx, cx]])
    rot_y = mk([[cy, zero, sy], [zero, one, zero], [-sy, zero, cy]])
    rot_z = mk([[cz, -sz, zero], [sz, cz, zero], [zero, zero, one]])
    scale_m = mk([[scale[:, 0], zero, zero], [zero, scale[:, 1], zero],
                  [zero, zero, scale[:, 2]]])
    t = np.tan(shear)
    shear_m = mk([[one, t[:, 0], t[:, 1]], [t[:, 2], one, t[:, 3]],
                  [t[:, 4], t[:, 5], one]])
    trans = translate[:, :, None].astype(ft)
    mat3 = (shear_m @ (scale_m @ (rot_z @ (rot_y @ rot_x)))).astype(ft)
    inv3 = np.linalg.inv(mat3).astype(ft)
    mat = np.concatenate([mat3, trans], axis=-1)
    inv_trans = np.matmul(-inv3, trans).astype(ft)
    inv_mat = np.concatenate([inv3, inv_trans], axis=-1)
    return mat, inv_mat


def default_cfg(N):
    if N >= 96:
        CW = 36
        cov = CW - 2
        bands = [-1]
        while bands[-1] + cov < N - 1:
            bands.append(min(bands[-1] + cov - 8, N - cov - 1))
        return dict(NP=16, SS=10, CW=CW, SP=128, Lseg=[16, 4], bands=bands)
    return dict(NP=16, SS=10, CW=N + 11, SP=32, Lseg=[8, 4], bands=[-1])


class Section:
    """Sampling geometry for one batch on one core (interleaved d)."""

    def __init__(self, matb, N, core):
        self.N, self.core = N, core
        self.SL = N // NCORES
        Am = matb[:, :3].astype(np.float64)
        t = matb[:, 3].astype(np.float64)
        self.cw = Am[:, 0].copy()
        self.ch = Am[:, 1].copy()
        self.cd = Am[:, 2].copy()
        s = Am.sum(1)
        self.bias = 0.5 * s - (N / 2.0) * s + (N / 2.0) * t + (N - 1) / 2.0
        self.dvals = np.arange(core, N, NCORES, dtype=np.float64)

    def coords(self):
        N, SL = self.N, self.SL
        D3 = self.dvals[:, None, None]
        W3 = np.arange(N, dtype=np.float64)[None, :, None]
        H3 = np.arange(N, dtype=np.float64)[None, None, :]
        out = []
        for k in range(3):
            v = (self.cw[k] * W3 + self.ch[k] * H3 + self.cd[k] * D3
                 + self.bias[k])
            out.append(np.broadcast_to(v, (SL, N, N)))
        return out


def plan_section(matb, N, core, Lseg, cfg):
    sec = Section(matb, N, core)
    SL = sec.SL
    NP, SS, CW, bands = cfg["NP"], cfg["SS"], cfg["CW"], cfg["bands"]
    ix, iy, iz = sec.coords()
    live = ((ix > -1 - 1e-4) & (ix < N + 1e-4)
            & (iy > -1 - 1e-4) & (iy < N + 1e-4)
            & (iz > -1 - 1e-4) & (iz < N + 1e-4))
    x0 = np.floor(ix).astype(np.int64)
    z0 = np.floor(iz).astype(np.int64)
    nseg_h = N // Lseg

    def segred(v, op):
        return op(v.reshape(SL, N, nseg_h, Lseg), axis=3)

    seg_live = segred(live, np.any)
    big = 10 * N
    z0m = segred(np.where(live, z0, big), np.min)
    z0M = segred(np.where(live, z0, -big), np.max)
    x0m = segred(np.where(live, x0, big), np.min)
    x0M = segred(np.where(live, x0, -big), np.max)
    pmin = np.clip(z0m - 1, 0, N - 1)
    pmax = np.clip(z0M + 2, 0, N - 1)
    xmin = np.clip(x0m - 1, -1, N)
    xmax = np.clip(x0M + 2, -1, N)
    j = np.maximum(0, -(-(pmax - (NP - 1)) // SS))
    if not np.all((SS * j <= pmin) | ~seg_live):
        raise RuntimeError("segment z-range doesn't fit slab window")
    band_id = np.full(j.shape, -1, np.int64)
    for bi, cs in enumerate(bands):
        fit = (xmin >= cs) & (xmax <= cs + CW - 1) & (band_id < 0)
        band_id = np.where(fit, bi, band_id)
    if not np.all((band_id >= 0) | ~seg_live):
        raise RuntimeError("segment doesn't fit any x band")
    return dict(sec=sec, seg_live=seg_live, j=j, band=band_id,
                nseg_h=nseg_h, Lseg=Lseg)


def plan_all(mat, N, cfg):
    """Returns (meta, percore, consts). Schedule identical across cores.

    Rounds are built by demand-sorted bucket packing: a bucket is a
    (batch, slab j, band) set of segments; each round has 8 group-slots,
    each slot = one bucket (window = slab j planes, cols of band).
    """
    SP = cfg["SP"]
    CHV = 16 * SP
    NP, SS, CW, bands = cfg["NP"], cfg["SS"], cfg["CW"], cfg["bands"]
    RW = N + 2
    NEQ = (N + 1) * CW
    SL = N // NCORES
    plans = [[plan_section(mat[b], N, c, cfg["Lseg"][b], cfg)
              for b in range(2)] for c in range(NCORES)]

    sched = []
    percore = [dict(FC=[], TKX=[], TKY=[], TKZ=[], OFF=[])
               for _ in range(NCORES)]
    total_pad = total_seg = 0
    scratch = 2 * SL * N * N
    for b in range(2):
        Lseg = cfg["Lseg"][b]
        spc = CHV // Lseg
        # buckets: (j, band) -> per-core segment index arrays
        keys = set()
        for c in range(NCORES):
            p = plans[c][b]
            sl = p["seg_live"]
            if sl.any():
                for jj, bb in zip(p["j"][sl], p["band"][sl]):
                    keys.add((int(jj), int(bb)))
        buckets = {}
        dem = {}
        for k in sorted(keys):
            jj, bb = k
            per = []
            mx = 0
            for c in range(NCORES):
                p = plans[c][b]
                idxs = np.argwhere(p["seg_live"] & (p["j"] == jj)
                                   & (p["band"] == bb))
                per.append(idxs)
                mx = max(mx, len(idxs))
            buckets[k] = per
            dem[k] = max(1, -(-mx // spc))
        # split buckets into pieces of <= QCAP chunks, then pack
        QCAP = 8
        pieces = []  # (key, qstart) with per-piece demand
        pdem = {}
        for k in sorted(buckets):
            for q in range(0, dem[k], QCAP):
                pieces.append((k, q))
                pdem[(k, q)] = min(QCAP, dem[k] - q)
        order = sorted(pieces, key=lambda kq: -pdem[kq])
        for i0 in range(0, len(order), 8):
            sl8 = order[i0:i0 + 8]
            nch = max(pdem[kq] for kq in sl8)
            slots = sl8 + [sl8[-1]] * (8 - len(sl8))
            use = [g < len(sl8) for g in range(8)]
            sched.append(dict(b=b, slots=[kq[0] for kq in slots],
                              nchunks=nch, Lseg=Lseg))
            total_pad += 8 * NCORES * nch * spc
            for c in range(NCORES):
                sec = plans[c][b]["sec"]
                ncol = nch * SP
                fc = np.zeros((128, ncol), np.float32)
                tkx = np.zeros((128, ncol), np.float32)
                tky = np.zeros((128, ncol), np.float32)
                tkz = np.full((128, ncol), 1e9, np.float32)
                off = np.full((8, nch * spc), scratch, np.uint32)
                for g in range(8):
                    if not use[g]:
                        continue
                    kk_, q_ = slots[g]
                    arr = buckets[kk_][c][q_ * spc:(q_ + nch) * spc]
                    nseg = len(arr)
                    total_seg += nseg
                    if nseg:
                        dl, w, sh = arr[:, 0], arr[:, 1], arr[:, 2]
                        h0 = sh * Lseg
                        dv = sec.dvals[dl]
                        lanes = np.arange(Lseg)
                        hh = (h0[:, None] + lanes[None, :]).reshape(-1)
                        ww = np.repeat(w, Lseg).astype(np.float64)
                        ddv = np.repeat(dv, Lseg)
                        i_lin = np.arange(nseg * Lseg)
                        pp = 16 * g + (i_lin % 16)
                        cc = i_lin // 16
                        fc[pp, cc] = hh
                        for k2, tk in ((0, tkx), (1, tky), (2, tkz)):
                            tk[pp, cc] = (sec.cw[k2] * ww + sec.cd[k2] * ddv
                                          + sec.bias[k2]).astype(np.float32)
                        off[g, :nseg] = (((b * SL + dl) * N + w) * N
                                         + h0).astype(np.uint32)
                pc = percore[c]
                pc["FC"].append(fc)
                pc["TKX"].append(tkx)
                pc["TKY"].append(tky)
                pc["TKZ"].append(tkz)
                pc["OFF"].append(off)

    # zero-fill offsets for unprocessed segments, per batch
    zo = [[], []]
    for b in range(2):
        Lseg = cfg["Lseg"][b]
        for c in range(NCORES):
            p = plans[c][b]
            idxs = np.argwhere(~p["seg_live"])
            if len(idxs):
                dl, w, sh = idxs[:, 0], idxs[:, 1], idxs[:, 2]
                o = (((b * SL + dl) * N + w) * N + sh * Lseg).astype(np.uint32)
            else:
                o = np.zeros(0, np.uint32)
            zo[b].append(o)
    zmax = [max(len(o) for o in zo[b]) for b in range(2)]
    zk = [max(1, -(-zmax[b] // 8)) for b in range(2)]
    for c in range(NCORES):
        zarr = []
        for b in range(2):
            o = np.full(8 * zk[b], scratch, np.uint32)
            o[: len(zo[b][c])] = zo[b][c]
            zarr.append(o.reshape(8, zk[b]))
        percore[c]["ZOFF"] = zarr

    diag = np.zeros((128, 16), np.float32)
    for p in range(128):
        diag[p, p % 16] = 1.0
    bones = np.zeros((128, 128), np.float32)
    for kk in range(128):
        bones[kk, 16 * (kk // 16):16 * (kk // 16) + 16] = 1.0
    bsum = np.zeros((128, 8), np.float32)
    for kk in range(128):
        bsum[kk, kk // 16] = 1.0
    nr = max(1, len(sched))
    plt = np.zeros((128, nr), np.float32)
    csc = np.zeros((128, nr), np.float32)
    for ri, sd in enumerate(sched):
        for g in range(8):
            jj, bb = sd["slots"][g]
            plt[16 * g:16 * g + 16, ri] = SS * jj + np.arange(16)
            csc[16 * g:16 * g + 16, ri] = CW - bands[bb]
    consts = dict(DIAG=diag, BONES=bones, BSUM=bsum, PLT=plt, CSC=csc)

    meta = dict(N=N, NP=NP, SS=SS, CW=CW, CWR=CW + 1, RW=RW, NEQ=NEQ,
                SP=SP, bands=bands, sched=sched, SL=SL, zk=zk,
                Lseg=cfg["Lseg"], scratch=scratch,
                pad_ratio=total_pad / max(total_seg, 1))
    meta["coef"] = [[float(np.float32(x)) for x in plans[0][b]["sec"].ch]
                    for b in range(2)]
    return meta, percore, consts


def build_program(meta, consts):
    """Build the SPMD bass program (identical for all cores)."""
    N, SL, SP = meta["N"], meta["SL"], meta["SP"]
    CW, CWR, RW, NEQ = meta["CW"], meta["CWR"], meta["RW"], meta["NEQ"]
    SS = meta["SS"]
    sched = meta["sched"]
    bands = meta["bands"]
    NI = 16 * SP                       # voxels per group per chunk
    PIECE = min(512, NI)
    NPIECE = NI // PIECE
    OUTN = 2 * SL * N * N + 64

    nc = bacc.Bacc("TRN2", target_bir_lowering=False, debug=False,
                   num_devices=NCORES)
    src_d = nc.dram_tensor("src", [2 * N * N * N], f32, kind="ExternalInput")
    tot_cols = sum(sd["nchunks"] * SP for sd in sched)
    fc_d = nc.dram_tensor("FC", [128, tot_cols], f32, kind="ExternalInput")
    tkx_d = nc.dram_tensor("TKX", [128, tot_cols], f32, kind="ExternalInput")
    tky_d = nc.dram_tensor("TKY", [128, tot_cols], f32, kind="ExternalInput")
    tkz_d = nc.dram_tensor("TKZ", [128, tot_cols], f32, kind="ExternalInput")
    diag_d = nc.dram_tensor("DIAG", [128, 16], f32, kind="ExternalInput")
    bones_d = nc.dram_tensor("BONES", [128, 128], f32, kind="ExternalInput")
    bsum_d = nc.dram_tensor("BSUM", [128, 8], f32, kind="ExternalInput")
    plt_d = nc.dram_tensor("PLT", [128, max(1, len(sched))], f32,
                           kind="ExternalInput")
    csc_d = nc.dram_tensor("CSC", [128, max(1, len(sched))], f32,
                           kind="ExternalInput")
    outs_d = nc.dram_tensor("outs", [8, tot_cols * 16], f32,
                            kind="ExternalOutput")

    def sub_ap(tl, pbase, pcount, free_off, free_pattern):
        pitch = tl.tensor.ap().ap[0][0]
        return bass.AP(tl.tensor, pbase * pitch + free_off,
                       [[pitch, pcount]] + free_pattern)

    with tile.TileContext(nc) as tc, ExitStack() as ctx:
        const_p = ctx.enter_context(tc.tile_pool(name="const", bufs=1))
        win_p = ctx.enter_context(tc.tile_pool(name="win", bufs=1))
        raw_p = ctx.enter_context(tc.tile_pool(name="raw", bufs=1))
        vm_p = ctx.enter_context(tc.tile_pool(name="vm", bufs=24))
        vin_p = ctx.enter_context(tc.tile_pool(name="vin", bufs=6))
        vidx_p = ctx.enter_context(tc.tile_pool(name="vidx", bufs=4))
        g_p = ctx.enter_context(tc.tile_pool(name="g", bufs=2))
        slot_p = ctx.enter_context(tc.tile_pool(name="slot", bufs=10))

        os_p = ctx.enter_context(tc.tile_pool(name="os", bufs=2))
        psA = ctx.enter_context(tc.tile_pool(name="psA", bufs=1, space="PSUM"))
        psB = ctx.enter_context(tc.tile_pool(name="psB", bufs=2, space="PSUM"))

        diag = const_p.tile([128, 16], f32)
        nc.sync.dma_start(diag[:], diag_d.ap())
        bones = const_p.tile([128, 128], f32)
        nc.sync.dma_start(bones[:], bones_d.ap())
        bsum = const_p.tile([128, 8], f32)
        nc.sync.dma_start(bsum[:], bsum_d.ap())
        plt = const_p.tile([128, max(1, len(sched))], f32)
        nc.sync.dma_start(plt[:], plt_d.ap())
        csc = const_p.tile([128, max(1, len(sched))], f32)
        nc.sync.dma_start(csc[:], csc_d.ap())

        col0 = 0
        for ri, sd in enumerate(sched):
            b = sd["b"]
            Lseg = sd["Lseg"]
            spc = NI // Lseg
            cxh, cyh, czh = (meta["coef"][b][k] for k in range(3))

            # ---- window fill (per-slot slab j / band cs) ----
            raw = raw_p.tile([128, RW * CWR], f32, tag="big")
            nc.scalar.memzero(raw[:])
            for g in range(8):
                jj, bb = sd["slots"][g]
                cs = bands[bb]
                u_lo = max(0, -cs)
                u_hi = min(CWR, N - cs)
                zbase = SS * jj
                nvalid = max(0, min(16, N - zbase))
                if nvalid > 0:
                    dst = sub_ap(raw, 16 * g, nvalid, CWR + u_lo,
                                 [[CWR, N], [1, u_hi - u_lo]])
                    srcap = bass.AP(src_d, b * N**3 + zbase * N * N
                                    + (cs + u_lo),
                                    [[N * N, nvalid], [N, N],
                                     [1, u_hi - u_lo]])
                    nc.sync.dma_start(dst, srcap)
            win = win_p.tile([128, NEQ * 4], f32, tag="win")
            for q, (dy, dx) in enumerate(((0, 0), (0, 1), (1, 0), (1, 1))):
                o = bass.AP(win.tensor, q,
                            [[win.tensor.ap().ap[0][0], 128],
                             [CW * 4, N + 1], [4, CW]])
                iw = bass.AP(raw.tensor, dy * CWR + dx,
                             [[raw.tensor.ap().ap[0][0], 128],
                              [CWR, N + 1], [1, CW]])
                nc.scalar.copy(o, iw)

            # ---- chunks ----
            for ch in range(sd["nchunks"]):
                c0 = col0 + ch * SP
                fct = vin_p.tile([128, SP], f32, tag="vin")
                nc.sync.dma_start(fct[:], fc_d.ap()[:, c0:c0 + SP])
                tkxt = vin_p.tile([128, SP], f32, tag="vin")
                nc.sync.dma_start(tkxt[:], tkx_d.ap()[:, c0:c0 + SP])
                tkyt = vin_p.tile([128, SP], f32, tag="vin")
                nc.sync.dma_start(tkyt[:], tky_d.ap()[:, c0:c0 + SP])
                tkzt = vin_p.tile([128, SP], f32, tag="vin")
                nc.sync.dma_start(tkzt[:], tkz_d.ap()[:, c0:c0 + SP])

                ixt = vm_p.tile([128, SP], f32, tag="vm")
                nc.vector.scalar_tensor_tensor(ixt[:], fct[:], cxh, tkxt[:],
                                               A.mult, A.add)
                iyt = vm_p.tile([128, SP], f32, tag="vm")
                nc.vector.scalar_tensor_tensor(iyt[:], fct[:], cyh, tkyt[:],
                                               A.mult, A.add)
                izt = vm_p.tile([128, SP], f32, tag="vm")
                nc.vector.scalar_tensor_tensor(izt[:], fct[:], czh, tkzt[:],
                                               A.mult, A.add)
                rx = vm_p.tile([128, SP], f32, tag="vm")
                nc.vector.tensor_scalar(rx[:], ixt[:], -0.5, MAGIC,
                                        A.add, A.add)
                xn = vm_p.tile([128, SP], f32, tag="vm")
                nc.vector.tensor_scalar(xn[:], rx[:], -1.0, MAGIC,
                                        A.mult, A.add)
                fracx = vm_p.tile([128, SP], f32, tag="vm")
                nc.vector.tensor_tensor(fracx[:], ixt[:], xn[:], A.add)
                ry = vm_p.tile([128, SP], f32, tag="vm")
                nc.vector.tensor_scalar(ry[:], iyt[:], -0.5, MAGIC,
                                        A.add, A.add)
                yn = vm_p.tile([128, SP], f32, tag="vm")
                nc.vector.tensor_scalar(yn[:], ry[:], -1.0, MAGIC,
                                        A.mult, A.add)
                fracy = vm_p.tile([128, SP], f32, tag="vm")
                nc.vector.tensor_tensor(fracy[:], iyt[:], yn[:], A.add)
                # oob -> push iz far away
                cxc = vm_p.tile([128, SP], f32, tag="vm")
                nc.vector.tensor_scalar(cxc[:], ixt[:], -1.0, float(N),
                                        A.max, A.min)
                ux = vm_p.tile([128, SP], f32, tag="vm")
                nc.vector.tensor_tensor(ux[:], ixt[:], cxc[:], A.subtract)
                izm = vm_p.tile([128, SP], f32, tag="vm")
                nc.vector.scalar_tensor_tensor(izm[:], ux[:], 1e6, izt[:],
                                               A.mult, A.add)
                cyc2 = vm_p.tile([128, SP], f32, tag="vm")
                nc.vector.tensor_scalar(cyc2[:], iyt[:], -1.0, float(N),
                                        A.max, A.min)
                uy = vm_p.tile([128, SP], f32, tag="vm")
                nc.vector.tensor_tensor(uy[:], iyt[:], cyc2[:], A.subtract)
                izm2 = vm_p.tile([128, SP], f32, tag="vm")
                nc.vector.scalar_tensor_tensor(izm2[:], uy[:], 1e6, izm[:],
                                               A.mult, A.add)
                # quad index
                t1 = vm_p.tile([128, SP], f32, tag="vm")
                nc.vector.tensor_scalar(t1[:], yn[:], -float(CW),
                                        csc[:, ri:ri + 1], A.mult, A.add)
                qx = vm_p.tile([128, SP], f32, tag="vm")
                nc.vector.scalar_tensor_tensor(qx[:], xn[:], -1.0, t1[:],
                                               A.mult, A.add)
                qc = vm_p.tile([128, SP], f32, tag="vm")
                nc.vector.tensor_scalar(qc[:], qx[:], 0.0, float(NEQ - 1),
                                        A.max, A.min)
                idxt = vidx_p.tile([128, SP], i16, tag="vidx")
                nc.vector.tensor_copy(idxt[:], qc[:])

                win3 = bass.AP(win.tensor, 0,
                               [[win.tensor.ap().ap[0][0], 128],
                                [4, NEQ], [1, 4]])
                for pi in range(NPIECE):
                    i0 = pi * PIECE
                    s0 = i0 // 16
                    scount = PIECE // 16
                    G = g_p.tile([128, PIECE, 4], f32, tag="G")
                    idxp = bass.AP(idxt.tensor, s0,
                                   [[idxt.tensor.ap().ap[0][0], 128],
                                    [1, scount]])
                    nc.gpsimd.ap_gather(G[:], win3, idxp, channels=128,
                                        num_elems=NEQ, d=4, num_idxs=PIECE)
                    # diag-expanded tiles
                    def dexp(vmt, tag):
                        dt_ = slot_p.tile([128, PIECE], f32, tag=tag)
                        vv = bass.AP(vmt.tensor, s0,
                                     [[vmt.tensor.ap().ap[0][0], 128],
                                      [1, scount], [0, 16]])
                        dd = bass.AP(diag.tensor, 0,
                                     [[diag.tensor.ap().ap[0][0], 128],
                                      [0, scount], [1, 16]])
                        nc.vector.tensor_tensor(dt_[:], vv, dd, A.mult)
                        return dt_

                    dfx = dexp(fracx, "st")
                    dfy = dexp(fracy, "st")
                    diz = dexp(izm2, "st")
                    fxr = psB.tile([128, PIECE], f32, tag="fxr")
                    nc.tensor.matmul(fxr[:], bones[:], dfx[:], start=True,
                                     stop=True)
                    fyr = psB.tile([128, PIECE], f32, tag="fyr")
                    nc.tensor.matmul(fyr[:], bones[:], dfy[:], start=True,
                                     stop=True)
                    izr = psB.tile([128, PIECE], f32, tag="izr")
                    nc.tensor.matmul(izr[:], bones[:], diz[:], start=True,
                                     stop=True)
                    # tent over planes
                    tzd = slot_p.tile([128, PIECE], f32, tag="st")
                    nc.vector.tensor_scalar(tzd[:], izr[:],
                                            plt[:, ri:ri + 1], None,
                                            A.subtract)
                    tza = slot_p.tile([128, PIECE], f32, tag="st")
                    nc.vector.tensor_scalar(tza[:], tzd[:], -1.0, 1.0,
                                            A.mult, A.add)
                    tzb = slot_p.tile([128, PIECE], f32, tag="st")
                    nc.vector.scalar_tensor_tensor(tzb[:], tzd[:], 1.0,
                                                   tza[:], A.add, A.min)
                    tzr = slot_p.tile([128, PIECE], f32, tag="st")
                    nc.scalar.activation(tzr[:], tzb[:], AF.Relu)
                    # x fold: pairs (0,1) and (2,3)
                    gst = G.tensor.ap().ap[0][0]
                    gG = lambda q: bass.AP(G.tensor, q,
                                           [[gst, 128], [4, PIECE]])
                    d1 = slot_p.tile([128, PIECE], f32, tag="st")
                    nc.vector.tensor_tensor(d1[:], gG(1), gG(0), A.subtract)
                    d2 = slot_p.tile([128, PIECE], f32, tag="st")
                    nc.vector.tensor_tensor(d2[:], gG(3), gG(2), A.subtract)
                    m1 = slot_p.tile([128, PIECE], f32, tag="st")
                    nc.vector.tensor_tensor(m1[:], d1[:], fxr[:], A.mult)
                    m2 = slot_p.tile([128, PIECE], f32, tag="st")
                    nc.vector.tensor_tensor(m2[:], d2[:], fxr[:], A.mult)
                    xa = slot_p.tile([128, PIECE], f32, tag="st")
                    nc.vector.tensor_tensor(xa[:], m1[:], gG(0), A.add)
                    xb = slot_p.tile([128, PIECE], f32, tag="st")
                    nc.vector.tensor_tensor(xb[:], m2[:], gG(2), A.add)
                    # y fold
                    dy_ = slot_p.tile([128, PIECE], f32, tag="st")
                    nc.vector.tensor_tensor(dy_[:], xb[:], xa[:], A.subtract)
                    my_ = slot_p.tile([128, PIECE], f32, tag="st")
                    nc.vector.tensor_tensor(my_[:], dy_[:], fyr[:], A.mult)
                    bt = slot_p.tile([128, PIECE], f32, tag="st")
                    nc.vector.tensor_tensor(bt[:], my_[:], xa[:], A.add)
                    zz = slot_p.tile([128, PIECE], f32, tag="st")
                    nc.vector.tensor_tensor(zz[:], bt[:], tzr[:], A.mult)
                    mm = psA.tile([8, PIECE], f32, tag="mm")
                    nc.tensor.matmul(mm[:], bsum[:], zz[:], start=True,
                                     stop=True)
                    outS = os_p.tile([8, PIECE], f32, tag="outS")
                    nc.scalar.copy(outS[:], mm[:])
                    oc0 = (c0 + pi * (PIECE // 16)) * 16
                    nc.sync.dma_start(outs_d.ap()[:, oc0:oc0 + PIECE],
                                      outS[:])
            col0 += sd["nchunks"] * SP

    nc.compile()
    return nc


def make_inputs(meta, percore, consts, src):
    """Build per-core in_maps. src: [2, 1, N, N, N] float32 full."""
    N, SL = meta["N"], meta["SL"]
    maps = []
    srcf = np.ascontiguousarray(src.reshape(2, N, N, N), np.float32)
    for c in range(NCORES):
        pc = percore[c]
        m = dict(
            src=srcf.reshape(-1),
            FC=np.concatenate(pc["FC"], 1) if pc["FC"] else np.zeros((128, 0), np.float32),
            TKX=np.concatenate(pc["TKX"], 1) if pc["TKX"] else np.zeros((128, 0), np.float32),
            TKY=np.concatenate(pc["TKY"], 1) if pc["TKY"] else np.zeros((128, 0), np.float32),
            TKZ=np.concatenate(pc["TKZ"], 1) if pc["TKZ"] else np.zeros((128, 0), np.float32),
            DIAG=consts["DIAG"], BONES=consts["BONES"], BSUM=consts["BSUM"],
            PLT=consts["PLT"], CSC=consts["CSC"],
        )
        maps.append(m)
    return maps


def assemble(meta, percore, results):
    """Place device-computed segment streams into the full output volume."""
    N, SL, SP = meta["N"], meta["SL"], meta["SP"]
    scratch = meta["scratch"]
    full = np.zeros((2, N, N, N), np.float32)
    for c in range(NCORES):
        outs = results[c]["outs"]          # [8, tot_cols*16]
        out_t = np.zeros(2 * SL * N * N + 64, np.float32)
        col0 = 0
        for ri, sd in enumerate(meta["sched"]):
            Lseg = sd["Lseg"]
            nch = sd["nchunks"]
            off = percore[c]["OFF"][ri]    # [8, nch*spc]
            blk = outs[:, col0 * 16:(col0 + nch * SP) * 16]
            vals = blk.reshape(8, -1, Lseg)
            offf = off.reshape(8, -1)
            m = offf != scratch
            tgt = (offf[m][:, None] + np.arange(Lseg)[None, :]).reshape(-1)
            out_t[tgt] = vals[m].reshape(-1)
            col0 += nch * SP
        o = out_t[: 2 * SL * N * N].reshape(2, SL, N, N)
        full[:, c::NCORES, :, :] = np.swapaxes(o, 2, 3)
    return full.reshape(2, 1, N, N, N)


# ======================================================================
# Harness entry point: kernel(**inputs) -> (warped, mat, inv_mat)
# ======================================================================
_CACHE = {}


def kernel(src, affine, scale, translate, shear):
    """Takes FULL inputs; shards across the 8 NeuronCores internally;
    returns the FULL outputs (warped volume, mat, inv_mat)."""
    import concourse.bass_utils as _bu

    src = np.asarray(src, np.float32)
    affine = np.asarray(affine, np.float32)
    scale = np.asarray(scale, np.float32)
    translate = np.asarray(translate, np.float32)
    shear = np.asarray(shear, np.float32)
    B, C, D, H, W = src.shape
    assert B == 2 and C == 1 and D == H == W, "kernel assumes (2,1,N,N,N)"
    N = D

    mat, inv_mat = compose_mats_np(affine, scale, translate, shear)

    key = (N, mat.tobytes())
    cfg = default_cfg(N)
    if key in _CACHE:
        meta, consts, nc, percore = _CACHE[key]
    else:
        meta, percore, consts = plan_all(mat, N, cfg)
        nc = build_program(meta, consts)
        _CACHE[key] = (meta, consts, nc, percore)

    in_maps = make_inputs(meta, percore, consts, src)
    res = _bu.run_bass_kernel_spmd(nc, in_maps, core_ids=list(range(NCORES)))
    warped = assemble(meta, percore, res.results)
    return warped, mat, inv_mat
